# revision 9
# baseline (speedup 1.0000x reference)
"""Trainium2 Bass kernel for the ChordDecoder GRU (batch 32768, 8 steps).

Strategy (pure data parallel, 8 cores x 4096 batch):
  - Feature-major (transposed) recurrence: hT [512, 4096] per core stays in
    SBUF; gates computed as W @ hT into PSUM with float32r matmuls.
  - z_in is folded on the host (M_gz = W_ih[:, 36:] @ z2dec_in_w); its
    per-batch contribution gz = M_gz @ z_chd^T (+ input biases) is computed
    once at init into DRAM (f32r) and streamed back each step, added into
    the gate PSUM via an identity matmul (1 accumulation instead of 2).
  - Heads are computed batch-major [128b, 48] with hT chunks as the
    stationary operand; argmax one-hots via reduce_max + is_ge with free-dim
    broadcast; the next-step token is transposed back to [36, b] with bf16
    PE transposes.
"""

import sys

sys.path.insert(0, "/opt/trn_rl_repo")

import numpy as np
import concourse.bacc as bacc
import concourse.mybir as mybir
from concourse.alu_op_type import AluOpType
from concourse.tile import TileContext
from concourse.bass_utils import run_bass_kernel_spmd

F32 = mybir.dt.float32
F32R = mybir.dt.float32r
BF16 = mybir.dt.bfloat16
AF = mybir.ActivationFunctionType
AX = mybir.AxisListType

BS = 32768
H = 512
Z = 256
TOK = 36
NSTEP = 8
NCORES = 8
BSH = BS // NCORES          # 4096 batch per core
NC = 512                    # batch columns per n-chunk
NCH = BSH // NC             # 8 n-chunks
BT = NC // 128              # 4 b-tiles per n-chunk
G3 = 3 * H                  # 1536 gate rows
MC = G3 // 128              # 12 gate row chunks
KH = H // 128               # 4 K-chunks for W_hh
KZ = Z // 128               # 2 K-chunks for z_chd

_BUILT = None


def _build():
    nc = bacc.Bacc()

    # ---- DRAM I/O ----
    z_chdT = nc.dram_tensor("z_chdT", [Z, BSH], F32R, kind="ExternalInput")
    tok0T = nc.dram_tensor("tok0T", [TOK, BSH], F32R, kind="ExternalInput")
    whhT = nc.dram_tensor("whhT", [H, G3], F32R, kind="ExternalInput")
    wtokT = nc.dram_tensor("wtokT", [TOK, G3], F32R, kind="ExternalInput")
    wgzT = nc.dram_tensor("wgzT", [Z, G3], F32R, kind="ExternalInput")
    whidT = nc.dram_tensor("whidT", [Z, H], F32R, kind="ExternalInput")
    wheadT = nc.dram_tensor("wheadT", [H, 48], F32, kind="ExternalInput")
    hbias = nc.dram_tensor("hbias", [1, 48], F32, kind="ExternalInput")
    brz = nc.dram_tensor("brz", [2 * H], F32, kind="ExternalInput")
    bhn = nc.dram_tensor("bhn", [H], F32, kind="ExternalInput")
    btanh = nc.dram_tensor("btanh", [H], F32, kind="ExternalInput")
    bh0 = nc.dram_tensor("bh0", [H], F32, kind="ExternalInput")
    cgz = nc.dram_tensor("cgz", [G3], F32, kind="ExternalInput")
    ident = nc.dram_tensor("ident", [128, 128], F32R, kind="ExternalInput")
    ident_b = nc.dram_tensor("ident_b", [128, 128], BF16, kind="ExternalInput")

    gz_d = nc.dram_tensor("gz_d", [MC, 128, BSH], F32R, kind="Internal")

    roots_o = nc.dram_tensor("roots_o", [NSTEP, BSH, 12], F32, kind="ExternalOutput")
    chroma_o = nc.dram_tensor("chroma_o", [NSTEP, BSH, 24], F32, kind="ExternalOutput")
    bass_o = nc.dram_tensor("bass_o", [NSTEP, BSH, 12], F32, kind="ExternalOutput")

    with TileContext(nc) as tc:
        with tc.tile_pool(name="const", bufs=1) as cp, \
             tc.tile_pool(name="psg", bufs=5, space="PSUM") as pg, \
             tc.tile_pool(name="psh", bufs=2, space="PSUM") as ph, \
             tc.tile_pool(name="pst", bufs=1, space="PSUM") as pt:

            # ---- persistent SBUF ----
            whh = cp.tile([128, KH, G3], F32R)
            wtk = cp.tile([TOK, G3], F32R)
            hT = cp.tile([128, KH, BSH], F32R)
            tokT = cp.tile([TOK, BSH], F32R)
            whd = cp.tile([128, KH, 48], F32)
            hbr = cp.tile([1, 48], F32)
            ones = cp.tile([1, 128], F32)
            idr = cp.tile([128, 128], F32R)
            idb = cp.tile([128, 128], BF16)
            brz_s = cp.tile([128, 8], F32)
            bhn_s = cp.tile([128, 4], F32)
            btn_s = cp.tile([128, 4], F32)
            bh0_s = cp.tile([128, 4], F32)
            cgz_s = cp.tile([128, MC], F32)

            nc.sync.dma_start(out=whd[:], in_=wheadT.rearrange("(c p) m -> p c m", p=128))
            nc.sync.dma_start(out=hbr[:], in_=hbias[:, :])
            nc.sync.dma_start(out=idb[:], in_=ident_b[:, :])
            nc.sync.dma_start(out=brz_s[:], in_=brz.rearrange("(m p) -> p m", p=128))
            nc.sync.dma_start(out=bhn_s[:], in_=bhn.rearrange("(m p) -> p m", p=128))
            nc.sync.dma_start(out=btn_s[:], in_=btanh.rearrange("(m p) -> p m", p=128))
            nc.sync.dma_start(out=bh0_s[:], in_=bh0.rearrange("(m p) -> p m", p=128))
            nc.sync.dma_start(out=cgz_s[:], in_=cgz.rearrange("(m p) -> p m", p=128))
            nc.vector.memset(ones[:], 1.0)

            # ---- init: direct f32r loads, h0, gz precompute ----
            nc.sync.dma_start(out=whh[:], in_=whhT.rearrange("(c p) m -> p c m", p=128))
            nc.sync.dma_start(out=wtk[:], in_=wtokT[:, :])
            nc.sync.dma_start(out=tokT[:], in_=tok0T[:, :])
            nc.sync.dma_start(out=idr[:], in_=ident[:, :])
            with tc.tile_pool(name="init", bufs=1) as ip, \
                 tc.tile_pool(name="gzsp", bufs=4) as gp:
                zch = ip.tile([128, KZ, BSH], F32R)
                wgz = ip.tile([128, KZ, G3], F32R)
                whi = ip.tile([128, KZ, H], F32R)
                nc.sync.dma_start(out=zch[:],
                                  in_=z_chdT.rearrange("(c p) b -> p c b", p=128))
                nc.sync.dma_start(out=wgz[:],
                                  in_=wgzT.rearrange("(c p) m -> p c m", p=128))
                nc.sync.dma_start(out=whi[:],
                                  in_=whidT.rearrange("(c p) m -> p c m", p=128))

                # h0T = whidT.T @ z_chdT + bh0
                for m in range(KH):
                    for ci in range(NCH):
                        sl = slice(ci * NC, (ci + 1) * NC)
                        ps = pg.tile([128, NC], F32, tag="gate")
                        for k in range(KZ):
                            nc.tensor.matmul(ps[:], whi[:, k, m * 128:(m + 1) * 128],
                                             zch[:, k, sl], start=(k == 0),
                                             stop=(k == KZ - 1))
                        nc.scalar.activation(hT[:, m, sl], ps[:], AF.Identity,
                                             bias=bh0_s[:, m:m + 1], scale=1.0)
                # gz = M_gz @ z_chdT + cgz -> DRAM (f32r)
                for m in range(MC):
                    for ci in range(NCH):
                        sl = slice(ci * NC, (ci + 1) * NC)
                        ps = pg.tile([128, NC], F32, tag="gate")
                        for k in range(KZ):
                            nc.tensor.matmul(ps[:], wgz[:, k, m * 128:(m + 1) * 128],
                                             zch[:, k, sl], start=(k == 0),
                                             stop=(k == KZ - 1))
                        gs = gp.tile([128, NC], F32R, tag="gzs")
                        nc.scalar.activation(gs[:], ps[:], AF.Identity,
                                             bias=cgz_s[:, m:m + 1], scale=1.0)
                        nc.sync.dma_start(out=gz_d[m, :, sl], in_=gs[:])

            # ---- step loop ----
            with tc.tile_pool(name="work", bufs=2) as wk, \
                 tc.tile_pool(name="nwk", bufs=1) as nwk, \
                 tc.tile_pool(name="hdp", bufs=2) as hdp, \
                 tc.tile_pool(name="gzp", bufs=2) as gzp:

                def gate_psum(m, ci, gz):
                    """PSUM accumulation of gate rows [128m, 128m+128) for
                    batch chunk ci: gh (K=512) + tok (K=36) + gz (streamed)."""
                    sl = slice(ci * NC, (ci + 1) * NC)
                    ms = slice(m * 128, (m + 1) * 128)
                    ps = pg.tile([128, NC], F32, tag="gate")
                    for k in range(KH):
                        nc.tensor.matmul(ps[:], whh[:, k, ms], hT[:, k, sl],
                                         start=(k == 0), stop=False)
                    nc.tensor.matmul(ps[:], wtk[:, ms], tokT[:, sl],
                                     start=False, stop=False)
                    nc.tensor.matmul(ps[:], idr[:], gz[:, m, :], start=False,
                                     stop=True)
                    return ps

                for t in range(NSTEP):
                    for ci in range(NCH):
                        sl = slice(ci * NC, (ci + 1) * NC)
                        # one batched gz load for all 12 gate row chunks
                        gz = gzp.tile([128, MC, NC], F32R, tag="gz")
                        nc.gpsimd.dma_start(
                            out=gz[:], in_=gz_d[:, :, sl].rearrange("m p b -> p m b"))
                        # r and n gates interleaved per row-chunk a:
                        # r rows 0:512 (m 0..3), n rows 1024:1536 (m 8..11)
                        nt = nwk.tile([128, 4, NC], F32, tag="n")
                        for a in range(4):
                            ps_r = gate_psum(a, ci, gz)
                            r_a = wk.tile([128, NC], F32, tag="r")
                            nc.scalar.activation(r_a[:], ps_r[:], AF.Sigmoid,
                                                 bias=brz_s[:, a:a + 1], scale=1.0)
                            m = 8 + a
                            sm = slice(m * 128, (m + 1) * 128)
                            ps_hn = pg.tile([128, NC], F32, tag="gate")
                            for k in range(KH):
                                nc.tensor.matmul(ps_hn[:], whh[:, k, sm], hT[:, k, sl],
                                                 start=(k == 0), stop=(k == KH - 1))
                            ps_in = pg.tile([128, NC], F32, tag="gate")
                            nc.tensor.matmul(ps_in[:], wtk[:, sm], tokT[:, sl],
                                             start=True, stop=False)
                            nc.tensor.matmul(ps_in[:], idr[:], gz[:, m, :],
                                             start=False, stop=True)
                            # npre = (gh_n + b_hh_n) * r ; += (tok+gz) ; tanh
                            npre = wk.tile([128, NC], F32, tag="npre")
                            nc.vector.scalar_tensor_tensor(
                                npre[:], ps_hn[:], bhn_s[:, a:a + 1], r_a[:],
                                AluOpType.add, AluOpType.mult)
                            nc.vector.tensor_tensor(npre[:], npre[:], ps_in[:],
                                                    AluOpType.add)
                            nc.scalar.activation(nt[:, a, :], npre[:], AF.Tanh,
                                                 bias=btn_s[:, a:a + 1], scale=1.0)
                        # z gates (rows 512:1024 -> m 4..7): ALL must read the
                        # old h before any h write below
                        zt = nwk.tile([128, 4, NC], F32, tag="z4")
                        for a in range(4):
                            ps_z = gate_psum(4 + a, ci, gz)
                            nc.scalar.activation(zt[:, a, :], ps_z[:], AF.Sigmoid,
                                                 bias=brz_s[:, 4 + a:5 + a], scale=1.0)
                        # h' = n + z * (h - n)
                        for a in range(4):
                            u_a = wk.tile([128, NC], F32, tag="r")
                            nc.gpsimd.tensor_tensor(u_a[:], hT[:, a, sl].bitcast(F32),
                                                    nt[:, a, :], AluOpType.subtract)
                            zd_a = wk.tile([128, NC], F32, tag="npre")
                            nc.vector.tensor_tensor(zd_a[:], zt[:, a, :], u_a[:],
                                                    AluOpType.mult)
                            nc.vector.tensor_tensor(hT[:, a, sl], nt[:, a, :], zd_a[:],
                                                    AluOpType.add)

                        # heads, batch-major [128, 48] per b-tile
                        HD = hdp.tile([128, BT, 48], F32, tag="hd")
                        for j in range(BT):
                            b0 = ci * NC + j * 128
                            psh = ph.tile([128, 48], F32, tag="hpsum")
                            for k in range(KH):
                                nc.tensor.matmul(psh[:],
                                                 hT[:, k, b0:b0 + 128].bitcast(F32),
                                                 whd[:, k, :], start=(k == 0),
                                                 stop=False)
                            nc.tensor.matmul(psh[:], ones[:, :], hbr[:, :],
                                             start=False, stop=True)
                            nc.scalar.copy(HD[:, j, :], psh[:])
                        # stream outputs (t-major DRAM layout)
                        nc.sync.dma_start(
                            out=roots_o[t, sl, :].rearrange("(j p) c -> p j c", p=128),
                            in_=HD[:, :, 0:12])
                        nc.sync.dma_start(
                            out=chroma_o[t, sl, :].rearrange("(j p) c -> p j c", p=128),
                            in_=HD[:, :, 12:36])
                        nc.sync.dma_start(
                            out=bass_o[t, sl, :].rearrange("(j p) c -> p j c", p=128),
                            in_=HD[:, :, 36:48])

                        if t == NSTEP - 1:
                            continue
                        # next token: one-hot(argmax root/bass) + chroma bits
                        tkb = hdp.tile([128, BT, TOK], BF16, tag="tkb")
                        mxr = hdp.tile([128, BT], F32, tag="mxr")
                        mxb = hdp.tile([128, BT], F32, tag="mxb")
                        nc.vector.tensor_reduce(mxr[:], HD[:, :, 0:12], AX.X,
                                                AluOpType.max)
                        nc.vector.tensor_reduce(mxb[:], HD[:, :, 36:48], AX.X,
                                                AluOpType.max)
                        nc.vector.tensor_tensor(
                            tkb[:, :, 0:12], HD[:, :, 0:12],
                            mxr[:].broadcast_to([128, BT, 12]), AluOpType.is_ge)
                        nc.vector.tensor_tensor(
                            tkb[:, :, 24:36], HD[:, :, 36:48],
                            mxb[:].broadcast_to([128, BT, 12]), AluOpType.is_ge)
                        pairs = HD[:, :, 12:36].rearrange(
                            "p j (c two) -> p j c two", two=2)
                        nc.vector.tensor_tensor(
                            tkb[:, :, 12:24], pairs[:, :, :, 1], pairs[:, :, :, 0],
                            AluOpType.is_gt)
                        # transpose token back to [36, NC] (bf16 PE transpose)
                        pstk = pt.tile([TOK, NC], BF16, tag="tk")
                        for j in range(BT):
                            nc.tensor.transpose(pstk[:, j * 128:(j + 1) * 128],
                                                tkb[:, j, :], idb[:])
                        nc.vector.tensor_copy(tokT[:, sl], pstk[:])

    nc.finalize()
    return nc


def _host_prep(inputs):
    f = np.float32
    W_ih = np.asarray(inputs["W_ih"], f)
    W_hh = np.asarray(inputs["W_hh"], f)
    b_ih = np.asarray(inputs["b_ih"], f)
    b_hh = np.asarray(inputs["b_hh"], f)
    z2in_w = np.asarray(inputs["z2dec_in_w"], f)
    z2in_b = np.asarray(inputs["z2dec_in_b"], f)
    z2hid_w = np.asarray(inputs["z2dec_hid_w"], f)
    z2hid_b = np.asarray(inputs["z2dec_hid_b"], f)
    W_tok = W_ih[:, :TOK]
    W_z = W_ih[:, TOK:]
    M_gz = (W_z @ z2in_w).astype(f)
    c_gz = (W_z @ z2in_b + b_ih).astype(f)

    wheads = np.concatenate([inputs["root_w"], inputs["chroma_w"],
                             inputs["bass_w"]], 0).astype(f)    # [48, 512]
    hb = np.concatenate([inputs["root_b"], inputs["chroma_b"],
                         inputs["bass_b"]], 0).astype(f)        # [48]

    import ml_dtypes
    common = {
        "whhT": np.ascontiguousarray(W_hh.T),
        "wtokT": np.ascontiguousarray(W_tok.T),
        "wgzT": np.ascontiguousarray(M_gz.T),
        "whidT": np.ascontiguousarray(z2hid_w.T),
        "wheadT": np.ascontiguousarray(wheads.T),
        "hbias": hb.reshape(1, 48),
        "brz": b_hh[:2 * H].copy(),
        "bhn": b_hh[2 * H:].copy(),
        "btanh": np.zeros(H, f),
        "bh0": z2hid_b,
        "cgz": c_gz,
        "ident": np.eye(128, dtype=f),
        "ident_b": np.eye(128).astype(ml_dtypes.bfloat16),
        "tok0T": np.ascontiguousarray(
            np.broadcast_to(np.asarray(inputs["init_input"], f)[:, None],
                            (TOK, BSH))),
    }
    z_chd = np.asarray(inputs["z_chd"], f)
    in_maps = []
    for i in range(NCORES):
        m = dict(common)
        m["z_chdT"] = np.ascontiguousarray(z_chd[i * BSH:(i + 1) * BSH, :].T)
        in_maps.append(m)
    return in_maps


def kernel(**inputs):
    global _BUILT
    if _BUILT is None:
        _BUILT = _build()
    nc = _BUILT
    in_maps = _host_prep(inputs)
    res = run_bass_kernel_spmd(nc, in_maps, list(range(NCORES))).results

    roots = np.concatenate([r["roots_o"].transpose(1, 0, 2) for r in res], 0)
    chroma = np.concatenate([r["chroma_o"].transpose(1, 0, 2) for r in res], 0)
    basses = np.concatenate([r["bass_o"].transpose(1, 0, 2) for r in res], 0)
    return (roots.astype(np.float32),
            chroma.reshape(BS, NSTEP, 12, 2).astype(np.float32),
            basses.astype(np.float32))


# revision 19
# speedup vs baseline: 4351.2397x; 4351.2397x over previous
"""Trainium2 Bass kernel for the ChordDecoder GRU (batch 32768, 8 steps).

Strategy (pure data parallel, 8 cores x 4096 batch):
  - Feature-major (transposed) recurrence: hT [512, 4096] per core stays in
    SBUF; gates computed as W @ hT into PSUM with float32r matmuls.
  - z_in is folded on the host (M_gz = W_ih[:, 36:] @ z2dec_in_w); its
    per-batch contribution gz = M_gz @ z_chd^T (+ input biases) is computed
    once at init into DRAM (f32r) and streamed back each step, added into
    the gate PSUM via an identity matmul (1 accumulation instead of 2).
  - Heads are computed batch-major [128b, 48] with hT chunks as the
    stationary operand; argmax one-hots via reduce_max + is_ge with free-dim
    broadcast; the next-step token is transposed back to [36, b] with bf16
    PE transposes.
"""

import sys

sys.path.insert(0, "/opt/trn_rl_repo")

import numpy as np
import concourse.bacc as bacc
import concourse.mybir as mybir
from concourse.alu_op_type import AluOpType
from concourse.tile import TileContext
from concourse.bass_utils import run_bass_kernel_spmd

F32 = mybir.dt.float32
F32R = mybir.dt.float32r
BF16 = mybir.dt.bfloat16
AF = mybir.ActivationFunctionType
AX = mybir.AxisListType

BS = 32768
H = 512
Z = 256
TOK = 36
NSTEP = 8
NCORES = 8
BSH = BS // NCORES          # 4096 batch per core
NC = 512                    # batch columns per n-chunk
NCH = BSH // NC             # 8 n-chunks
BT = NC // 128              # 4 b-tiles per n-chunk
G3 = 3 * H                  # 1536 gate rows
MC = G3 // 128              # 12 gate row chunks
KH = H // 128               # 4 K-chunks for W_hh
KZ = Z // 128               # 2 K-chunks for z_chd

_BUILT = None


def _build():
    nc = bacc.Bacc()

    # ---- DRAM I/O ----
    z_chdT = nc.dram_tensor("z_chdT", [Z, BSH], F32R, kind="ExternalInput")
    tok0T = nc.dram_tensor("tok0T", [TOK, BSH], F32R, kind="ExternalInput")
    whhT = nc.dram_tensor("whhT", [H, G3], F32R, kind="ExternalInput")
    wtokT = nc.dram_tensor("wtokT", [TOK, G3], F32R, kind="ExternalInput")
    wgzT = nc.dram_tensor("wgzT", [Z, G3], F32R, kind="ExternalInput")
    whidT = nc.dram_tensor("whidT", [Z, H], F32R, kind="ExternalInput")
    wheadT = nc.dram_tensor("wheadT", [H, 48], F32R, kind="ExternalInput")
    hbias = nc.dram_tensor("hbias", [1, 48], F32, kind="ExternalInput")
    brz = nc.dram_tensor("brz", [2 * H], F32, kind="ExternalInput")
    bhn = nc.dram_tensor("bhn", [H], F32, kind="ExternalInput")
    btanh = nc.dram_tensor("btanh", [H], F32, kind="ExternalInput")
    bh0 = nc.dram_tensor("bh0", [H], F32, kind="ExternalInput")
    cgz = nc.dram_tensor("cgz", [G3], F32, kind="ExternalInput")
    ident = nc.dram_tensor("ident", [128, 128], F32R, kind="ExternalInput")
    ident_b = nc.dram_tensor("ident_b", [128, 128], BF16, kind="ExternalInput")

    gz_d = nc.dram_tensor("gz_d", [MC, 128, BSH], F32R, kind="Internal")

    roots_o = nc.dram_tensor("roots_o", [NSTEP, BSH, 12], F32, kind="ExternalOutput")
    chroma_o = nc.dram_tensor("chroma_o", [NSTEP, BSH, 24], F32, kind="ExternalOutput")
    bass_o = nc.dram_tensor("bass_o", [NSTEP, BSH, 12], F32, kind="ExternalOutput")

    with TileContext(nc) as tc:
        with tc.tile_pool(name="const", bufs=1) as cp, \
             tc.tile_pool(name="psg", bufs=5, space="PSUM") as pg, \
             tc.tile_pool(name="psh", bufs=1, space="PSUM") as ph, \
             tc.tile_pool(name="pst", bufs=1, space="PSUM") as pt:

            # ---- persistent SBUF ----
            whh = cp.tile([128, KH, G3], F32R)
            wtk = cp.tile([TOK, G3], F32R)
            hT = cp.tile([128, KH, BSH], F32R)
            tokT = cp.tile([TOK, BSH], F32R)
            whd = cp.tile([128, KH, 48], F32R)
            hb48 = cp.tile([48, 1], F32)
            idr = cp.tile([128, 128], F32R)
            idb = cp.tile([128, 128], BF16)
            brz_s = cp.tile([128, 8], F32)
            bhn_s = cp.tile([128, 4], F32)
            btn_s = cp.tile([128, 4], F32)
            bh0_s = cp.tile([128, 4], F32)
            cgz_s = cp.tile([128, MC], F32)

            nc.sync.dma_start(out=whd[:], in_=wheadT.rearrange("(c p) m -> p c m", p=128))
            nc.sync.dma_start(out=hb48[:], in_=hbias.rearrange("one c -> c one"))
            nc.sync.dma_start(out=idb[:], in_=ident_b[:, :])
            nc.sync.dma_start(out=brz_s[:], in_=brz.rearrange("(m p) -> p m", p=128))
            nc.sync.dma_start(out=bhn_s[:], in_=bhn.rearrange("(m p) -> p m", p=128))
            nc.sync.dma_start(out=btn_s[:], in_=btanh.rearrange("(m p) -> p m", p=128))
            nc.sync.dma_start(out=bh0_s[:], in_=bh0.rearrange("(m p) -> p m", p=128))
            nc.sync.dma_start(out=cgz_s[:], in_=cgz.rearrange("(m p) -> p m", p=128))

            # ---- init: direct f32r loads, h0, gz precompute ----
            nc.gpsimd.dma_start(out=whh[:], in_=whhT.rearrange("(c p) m -> p c m", p=128))
            nc.gpsimd.dma_start(out=wtk[:], in_=wtokT[:, :])
            nc.gpsimd.dma_start(out=tokT[:], in_=tok0T[:, :])
            nc.gpsimd.dma_start(out=idr[:], in_=ident[:, :])
            with tc.tile_pool(name="init", bufs=1) as ip, \
                 tc.tile_pool(name="gzsp", bufs=4) as gp:
                zch = ip.tile([128, KZ, BSH], F32R)
                wgz = ip.tile([128, KZ, G3], F32R)
                whi = ip.tile([128, KZ, H], F32R)
                nc.sync.dma_start(out=whi[:],
                                  in_=whidT.rearrange("(c p) m -> p c m", p=128))
                for q in range(4):
                    qs = slice(q * (BSH // 4), (q + 1) * (BSH // 4))
                    nc.sync.dma_start(
                        out=zch[:, :, qs],
                        in_=z_chdT.rearrange("(c p) b -> p c b", p=128)[:, :, qs])
                nc.sync.dma_start(out=wgz[:],
                                  in_=wgzT.rearrange("(c p) m -> p c m", p=128))

                # h0T = whidT.T @ z_chdT + bh0
                for m in range(KH):
                    for ci in range(NCH):
                        sl = slice(ci * NC, (ci + 1) * NC)
                        ps = pg.tile([128, NC], F32, tag="gate")
                        for k in range(KZ):
                            nc.tensor.matmul(ps[:], whi[:, k, m * 128:(m + 1) * 128],
                                             zch[:, k, sl], start=(k == 0),
                                             stop=(k == KZ - 1))
                        nc.scalar.activation(hT[:, m, sl], ps[:], AF.Identity,
                                             bias=bh0_s[:, m:m + 1], scale=1.0)
                # gz = M_gz @ z_chdT + cgz -> DRAM (f32r)
                for m in range(MC):
                    for ci in range(NCH):
                        sl = slice(ci * NC, (ci + 1) * NC)
                        ps = pg.tile([128, NC], F32, tag="gate")
                        for k in range(KZ):
                            nc.tensor.matmul(ps[:], wgz[:, k, m * 128:(m + 1) * 128],
                                             zch[:, k, sl], start=(k == 0),
                                             stop=(k == KZ - 1))
                        gs = gp.tile([128, NC], F32R, tag="gzs")
                        nc.scalar.activation(gs[:], ps[:], AF.Identity,
                                             bias=cgz_s[:, m:m + 1], scale=1.0)
                        eng = nc.sync if ci % 2 == 0 else nc.gpsimd
                        eng.dma_start(out=gz_d[m, :, sl], in_=gs[:])

            # ---- step loop ----
            with tc.tile_pool(name="work", bufs=2) as wk, \
                 tc.tile_pool(name="nwk", bufs=1) as nwk, \
                 tc.tile_pool(name="hdp", bufs=2) as hdp, \
                 tc.tile_pool(name="gzp", bufs=2) as gzp:

                def gate_psum(m, ci, gz=None):
                    """PSUM accumulation of gate rows [128m, 128m+128) for
                    batch chunk ci: gh (K=512) + tok (K=36) [+ gz via identity
                    matmul when given; r/z gates add gz on DVE instead]."""
                    sl = slice(ci * NC, (ci + 1) * NC)
                    ms = slice(m * 128, (m + 1) * 128)
                    ps = pg.tile([128, NC], F32, tag="gate")
                    for k in range(KH):
                        nc.tensor.matmul(ps[:], whh[:, k, ms], hT[:, k, sl],
                                         start=(k == 0), stop=False)
                    nc.tensor.matmul(ps[:], wtk[:, ms], tokT[:, sl],
                                     start=False, stop=(gz is None))
                    if gz is not None:
                        nc.tensor.matmul(ps[:], idr[:], gz[:, m, :], start=False,
                                         stop=True)
                    return ps

                for t in range(NSTEP):
                    for ci in range(NCH):
                        sl = slice(ci * NC, (ci + 1) * NC)
                        # one batched gz load for all 12 gate row chunks
                        gz = gzp.tile([128, MC, NC], F32R, tag="gz")
                        nc.gpsimd.dma_start(
                            out=gz[:], in_=gz_d[:, :, sl].rearrange("m p b -> p m b"))
                        # r and n gates interleaved per row-chunk a:
                        # r rows 0:512 (m 0..3), n rows 1024:1536 (m 8..11)
                        nt = nwk.tile([128, 4, NC], F32, tag="n")
                        for a in range(4):
                            ps_r = gate_psum(a, ci, gz)
                            r_a = wk.tile([128, NC], F32, tag="r")
                            nc.scalar.activation(r_a[:], ps_r[:], AF.Sigmoid,
                                                 bias=brz_s[:, a:a + 1], scale=1.0)
                            m = 8 + a
                            sm = slice(m * 128, (m + 1) * 128)
                            ps_hn = pg.tile([128, NC], F32, tag="gate")
                            for k in range(KH):
                                nc.tensor.matmul(ps_hn[:], whh[:, k, sm], hT[:, k, sl],
                                                 start=(k == 0), stop=(k == KH - 1))
                            ps_in = pg.tile([128, NC], F32, tag="gate")
                            nc.tensor.matmul(ps_in[:], wtk[:, sm], tokT[:, sl],
                                             start=True, stop=False)
                            nc.tensor.matmul(ps_in[:], idr[:], gz[:, m, :],
                                             start=False, stop=True)
                            # npre = (gh_n + b_hh_n) * r ; += (tok+gz) ; tanh
                            npre = wk.tile([128, NC], F32, tag="npre")
                            nc.vector.scalar_tensor_tensor(
                                npre[:], ps_hn[:], bhn_s[:, a:a + 1], r_a[:],
                                AluOpType.add, AluOpType.mult)
                            nc.vector.tensor_tensor(npre[:], npre[:], ps_in[:],
                                                    AluOpType.add)
                            nc.scalar.activation(nt[:, a, :], npre[:], AF.Tanh,
                                                 bias=btn_s[:, a:a + 1], scale=1.0)
                        # z gates (rows 512:1024 -> m 4..7): ALL must read the
                        # old h before any h write below
                        zt = nwk.tile([128, 4, NC], F32, tag="z4")
                        for a in range(4):
                            ps_z = gate_psum(4 + a, ci, gz)
                            nc.scalar.activation(zt[:, a, :], ps_z[:], AF.Sigmoid,
                                                 bias=brz_s[:, 4 + a:5 + a], scale=1.0)
                        # h' = n + z * (h - n)
                        for a in range(4):
                            u_a = wk.tile([128, NC], F32, tag="r")
                            nc.gpsimd.tensor_tensor(u_a[:], hT[:, a, sl].bitcast(F32),
                                                    nt[:, a, :], AluOpType.subtract)
                            zd_a = wk.tile([128, NC], F32, tag="npre")
                            nc.vector.tensor_tensor(zd_a[:], zt[:, a, :], u_a[:],
                                                    AluOpType.mult)
                            nc.vector.tensor_tensor(hT[:, a, sl], nt[:, a, :], zd_a[:],
                                                    AluOpType.add)

                        # heads: transposed [48, NC] with whd stationary
                        # (f32r full-rate N=512), bias per-partition, then PE
                        # transposes back to batch-major [128, 48] tiles
                        pst_h = ph.tile([48, NC], F32, tag="hpsum")
                        for k in range(KH):
                            nc.tensor.matmul(pst_h[:], whd[:, k, :],
                                             hT[:, k, sl], start=(k == 0),
                                             stop=(k == KH - 1))
                        hds = hdp.tile([48, NC], F32, tag="hds")
                        nc.scalar.activation(hds[:], pst_h[:], AF.Identity,
                                             bias=hb48[:, 0:1], scale=1.0)
                        psb = ph.tile([128, BT, 48], F32, tag="hpsum2")
                        for j in range(BT):
                            nc.tensor.transpose(psb[:, j, :],
                                                hds[:, j * 128:(j + 1) * 128],
                                                idr[0:48, 0:48].bitcast(F32))
                        HD = hdp.tile([128, BT, 48], F32, tag="hd")
                        nc.scalar.copy(HD[:], psb[:])
                        # stream outputs (t-major DRAM layout)
                        nc.sync.dma_start(
                            out=roots_o[t, sl, :].rearrange("(j p) c -> p j c", p=128),
                            in_=HD[:, :, 0:12])
                        nc.sync.dma_start(
                            out=chroma_o[t, sl, :].rearrange("(j p) c -> p j c", p=128),
                            in_=HD[:, :, 12:36])
                        nc.sync.dma_start(
                            out=bass_o[t, sl, :].rearrange("(j p) c -> p j c", p=128),
                            in_=HD[:, :, 36:48])

                        if t == NSTEP - 1:
                            continue
                        # next token: one-hot(argmax root/bass) + chroma bits
                        tkb = hdp.tile([128, BT, TOK], BF16, tag="tkb")
                        mxr = hdp.tile([128, BT], F32, tag="mxr")
                        mxb = hdp.tile([128, BT], F32, tag="mxb")
                        nc.vector.tensor_reduce(mxr[:], HD[:, :, 0:12], AX.X,
                                                AluOpType.max)
                        nc.vector.tensor_reduce(mxb[:], HD[:, :, 36:48], AX.X,
                                                AluOpType.max)
                        nc.vector.tensor_tensor(
                            tkb[:, :, 0:12], HD[:, :, 0:12],
                            mxr[:].broadcast_to([128, BT, 12]), AluOpType.is_ge)
                        nc.vector.tensor_tensor(
                            tkb[:, :, 24:36], HD[:, :, 36:48],
                            mxb[:].broadcast_to([128, BT, 12]), AluOpType.is_ge)
                        pairs = HD[:, :, 12:36].rearrange(
                            "p j (c two) -> p j c two", two=2)
                        nc.vector.tensor_tensor(
                            tkb[:, :, 12:24], pairs[:, :, :, 1], pairs[:, :, :, 0],
                            AluOpType.is_gt)
                        # transpose token back to [36, NC] (bf16 PE transpose)
                        pstk = pt.tile([TOK, NC], BF16, tag="tk")
                        for j in range(BT):
                            nc.tensor.transpose(pstk[:, j * 128:(j + 1) * 128],
                                                tkb[:, j, :], idb[:])
                        nc.vector.tensor_copy(tokT[:, sl], pstk[:])

    nc.finalize()
    return nc


def _host_prep(inputs):
    f = np.float32
    W_ih = np.asarray(inputs["W_ih"], f)
    W_hh = np.asarray(inputs["W_hh"], f)
    b_ih = np.asarray(inputs["b_ih"], f)
    b_hh = np.asarray(inputs["b_hh"], f)
    z2in_w = np.asarray(inputs["z2dec_in_w"], f)
    z2in_b = np.asarray(inputs["z2dec_in_b"], f)
    z2hid_w = np.asarray(inputs["z2dec_hid_w"], f)
    z2hid_b = np.asarray(inputs["z2dec_hid_b"], f)
    W_tok = W_ih[:, :TOK]
    W_z = W_ih[:, TOK:]
    M_gz = (W_z @ z2in_w).astype(f)
    c_gz = (W_z @ z2in_b + b_ih).astype(f)

    wheads = np.concatenate([inputs["root_w"], inputs["chroma_w"],
                             inputs["bass_w"]], 0).astype(f)    # [48, 512]
    hb = np.concatenate([inputs["root_b"], inputs["chroma_b"],
                         inputs["bass_b"]], 0).astype(f)        # [48]

    import ml_dtypes
    common = {
        "whhT": np.ascontiguousarray(W_hh.T),
        "wtokT": np.ascontiguousarray(W_tok.T),
        "wgzT": np.ascontiguousarray(M_gz.T),
        "whidT": np.ascontiguousarray(z2hid_w.T),
        "wheadT": np.ascontiguousarray(wheads.T),
        "hbias": hb.reshape(1, 48),
        "brz": b_hh[:2 * H].copy(),
        "bhn": b_hh[2 * H:].copy(),
        "btanh": np.zeros(H, f),
        "bh0": z2hid_b,
        "cgz": c_gz,
        "ident": np.eye(128, dtype=f),
        "ident_b": np.eye(128).astype(ml_dtypes.bfloat16),
        "tok0T": np.ascontiguousarray(
            np.broadcast_to(np.asarray(inputs["init_input"], f)[:, None],
                            (TOK, BSH))),
    }
    z_chd = np.asarray(inputs["z_chd"], f)
    in_maps = []
    for i in range(NCORES):
        m = dict(common)
        m["z_chdT"] = np.ascontiguousarray(z_chd[i * BSH:(i + 1) * BSH, :].T)
        in_maps.append(m)
    return in_maps


def kernel(**inputs):
    global _BUILT
    if _BUILT is None:
        _BUILT = _build()
    nc = _BUILT
    in_maps = _host_prep(inputs)
    res = run_bass_kernel_spmd(nc, in_maps, list(range(NCORES))).results

    roots = np.concatenate([r["roots_o"].transpose(1, 0, 2) for r in res], 0)
    chroma = np.concatenate([r["chroma_o"].transpose(1, 0, 2) for r in res], 0)
    basses = np.concatenate([r["bass_o"].transpose(1, 0, 2) for r in res], 0)
    return (roots.astype(np.float32),
            chroma.reshape(BS, NSTEP, 12, 2).astype(np.float32),
            basses.astype(np.float32))


# revision 20
# speedup vs baseline: 4563.0974x; 1.0487x over previous
"""Trainium2 Bass kernel for the ChordDecoder GRU (batch 32768, 8 steps).

Strategy (pure data parallel, 8 cores x 4096 batch):
  - Feature-major (transposed) recurrence: hT [512, 4096] per core stays in
    SBUF; gates computed as W @ hT into PSUM with float32r matmuls.
  - z_in is folded on the host (M_gz = W_ih[:, 36:] @ z2dec_in_w); its
    per-batch contribution gz = M_gz @ z_chd^T (+ input biases) is computed
    once at init into DRAM (f32r) and streamed back each step, added into
    the gate PSUM via an identity matmul (1 accumulation instead of 2).
  - Heads are computed batch-major [128b, 48] with hT chunks as the
    stationary operand; argmax one-hots via reduce_max + is_ge with free-dim
    broadcast; the next-step token is transposed back to [36, b] with bf16
    PE transposes.
"""

import sys

sys.path.insert(0, "/opt/trn_rl_repo")

import numpy as np
import concourse.bacc as bacc
import concourse.mybir as mybir
from concourse.alu_op_type import AluOpType
from concourse.tile import TileContext
from concourse.bass_utils import run_bass_kernel_spmd

F32 = mybir.dt.float32
F32R = mybir.dt.float32r
BF16 = mybir.dt.bfloat16
AF = mybir.ActivationFunctionType
AX = mybir.AxisListType

BS = 32768
H = 512
Z = 256
TOK = 36
NSTEP = 8
NCORES = 8
BSH = BS // NCORES          # 4096 batch per core
NC = 512                    # batch columns per n-chunk
NCH = BSH // NC             # 8 n-chunks
BT = NC // 128              # 4 b-tiles per n-chunk
G3 = 3 * H                  # 1536 gate rows
MC = G3 // 128              # 12 gate row chunks
KH = H // 128               # 4 K-chunks for W_hh
KZ = Z // 128               # 2 K-chunks for z_chd

_BUILT = None


def _build():
    nc = bacc.Bacc()

    # ---- DRAM I/O ----
    z_chdT = nc.dram_tensor("z_chdT", [Z, BSH], F32R, kind="ExternalInput")
    tok0T = nc.dram_tensor("tok0T", [TOK, BSH], F32R, kind="ExternalInput")
    whhT = nc.dram_tensor("whhT", [H, G3], F32R, kind="ExternalInput")
    wtokT = nc.dram_tensor("wtokT", [TOK, G3], F32R, kind="ExternalInput")
    wgzT = nc.dram_tensor("wgzT", [Z, G3], F32R, kind="ExternalInput")
    whidT = nc.dram_tensor("whidT", [Z, H], F32R, kind="ExternalInput")
    wheadT = nc.dram_tensor("wheadT", [H, 48], F32R, kind="ExternalInput")
    hbias = nc.dram_tensor("hbias", [1, 48], F32, kind="ExternalInput")
    brz = nc.dram_tensor("brz", [2 * H], F32, kind="ExternalInput")
    bhn = nc.dram_tensor("bhn", [H], F32, kind="ExternalInput")
    btanh = nc.dram_tensor("btanh", [H], F32, kind="ExternalInput")
    bh0 = nc.dram_tensor("bh0", [H], F32, kind="ExternalInput")
    cgz = nc.dram_tensor("cgz", [G3], F32, kind="ExternalInput")
    ident = nc.dram_tensor("ident", [128, 128], F32R, kind="ExternalInput")
    ident_b = nc.dram_tensor("ident_b", [128, 128], BF16, kind="ExternalInput")

    gz_d = nc.dram_tensor("gz_d", [MC, 128, BSH], F32R, kind="Internal")

    roots_o = nc.dram_tensor("roots_o", [NSTEP, BSH, 12], F32, kind="ExternalOutput")
    chroma_o = nc.dram_tensor("chroma_o", [NSTEP, BSH, 24], F32, kind="ExternalOutput")
    bass_o = nc.dram_tensor("bass_o", [NSTEP, BSH, 12], F32, kind="ExternalOutput")

    with TileContext(nc) as tc:
        with tc.tile_pool(name="const", bufs=1) as cp, \
             tc.tile_pool(name="psg", bufs=5, space="PSUM") as pg, \
             tc.tile_pool(name="psh", bufs=1, space="PSUM") as ph, \
             tc.tile_pool(name="pst", bufs=1, space="PSUM") as pt:

            # ---- persistent SBUF ----
            whh = cp.tile([128, KH, G3], F32R)
            wtk = cp.tile([TOK, G3], F32R)
            hT = cp.tile([128, KH, BSH], F32R)
            tokT = cp.tile([TOK, BSH], F32R)
            whd = cp.tile([128, KH, 48], F32R)
            hb48 = cp.tile([48, 1], F32)
            idr = cp.tile([128, 128], F32R)
            idb = cp.tile([128, 128], BF16)
            brz_s = cp.tile([128, 8], F32)
            bhn_s = cp.tile([128, 4], F32)
            btn_s = cp.tile([128, 4], F32)
            bh0_s = cp.tile([128, 4], F32)
            cgz_s = cp.tile([128, MC], F32)

            nc.sync.dma_start(out=whd[:], in_=wheadT.rearrange("(c p) m -> p c m", p=128))
            nc.sync.dma_start(out=hb48[:], in_=hbias.rearrange("one c -> c one"))
            nc.sync.dma_start(out=idb[:], in_=ident_b[:, :])
            nc.sync.dma_start(out=brz_s[:], in_=brz.rearrange("(m p) -> p m", p=128))
            nc.sync.dma_start(out=bhn_s[:], in_=bhn.rearrange("(m p) -> p m", p=128))
            nc.sync.dma_start(out=btn_s[:], in_=btanh.rearrange("(m p) -> p m", p=128))
            nc.sync.dma_start(out=bh0_s[:], in_=bh0.rearrange("(m p) -> p m", p=128))
            nc.sync.dma_start(out=cgz_s[:], in_=cgz.rearrange("(m p) -> p m", p=128))

            # ---- init: direct f32r loads, h0, gz precompute ----
            nc.gpsimd.dma_start(out=whh[:], in_=whhT.rearrange("(c p) m -> p c m", p=128))
            nc.gpsimd.dma_start(out=wtk[:], in_=wtokT[:, :])
            nc.gpsimd.dma_start(out=tokT[:], in_=tok0T[:, :])
            nc.gpsimd.dma_start(out=idr[:], in_=ident[:, :])
            with tc.tile_pool(name="init", bufs=1) as ip, \
                 tc.tile_pool(name="gzsp", bufs=4) as gp:
                zch = ip.tile([128, KZ, BSH], F32R)
                wgz = ip.tile([128, KZ, G3], F32R)
                whi = ip.tile([128, KZ, H], F32R)
                nc.sync.dma_start(out=whi[:],
                                  in_=whidT.rearrange("(c p) m -> p c m", p=128))
                for q in range(4):
                    qs = slice(q * (BSH // 4), (q + 1) * (BSH // 4))
                    nc.sync.dma_start(
                        out=zch[:, :, qs],
                        in_=z_chdT.rearrange("(c p) b -> p c b", p=128)[:, :, qs])
                nc.sync.dma_start(out=wgz[:],
                                  in_=wgzT.rearrange("(c p) m -> p c m", p=128))

                # h0T = whidT.T @ z_chdT + bh0
                for m in range(KH):
                    for ci in range(NCH):
                        sl = slice(ci * NC, (ci + 1) * NC)
                        ps = pg.tile([128, NC], F32, tag="gate")
                        for k in range(KZ):
                            nc.tensor.matmul(ps[:], whi[:, k, m * 128:(m + 1) * 128],
                                             zch[:, k, sl], start=(k == 0),
                                             stop=(k == KZ - 1))
                        nc.scalar.activation(hT[:, m, sl], ps[:], AF.Identity,
                                             bias=bh0_s[:, m:m + 1], scale=1.0)
                # gz = M_gz @ z_chdT + cgz -> DRAM (f32r)
                for m in range(MC):
                    for ci in range(NCH):
                        sl = slice(ci * NC, (ci + 1) * NC)
                        ps = pg.tile([128, NC], F32, tag="gate")
                        for k in range(KZ):
                            nc.tensor.matmul(ps[:], wgz[:, k, m * 128:(m + 1) * 128],
                                             zch[:, k, sl], start=(k == 0),
                                             stop=(k == KZ - 1))
                        gs = gp.tile([128, NC], F32R, tag="gzs")
                        nc.scalar.activation(gs[:], ps[:], AF.Identity,
                                             bias=cgz_s[:, m:m + 1], scale=1.0)
                        eng = nc.sync if ci % 2 == 0 else nc.gpsimd
                        eng.dma_start(out=gz_d[m, :, sl], in_=gs[:])

            # ---- step loop ----
            with tc.tile_pool(name="work", bufs=2) as wk, \
                 tc.tile_pool(name="nwk", bufs=1) as nwk, \
                 tc.tile_pool(name="hdp", bufs=2) as hdp, \
                 tc.tile_pool(name="gzp", bufs=2) as gzp:

                def gate_psum(m, ci, gz=None):
                    """PSUM accumulation of gate rows [128m, 128m+128) for
                    batch chunk ci: gh (K=512) + tok (K=36) [+ gz via identity
                    matmul when given; r/z gates add gz on DVE instead]."""
                    sl = slice(ci * NC, (ci + 1) * NC)
                    ms = slice(m * 128, (m + 1) * 128)
                    ps = pg.tile([128, NC], F32, tag="gate")
                    for k in range(KH):
                        nc.tensor.matmul(ps[:], whh[:, k, ms], hT[:, k, sl],
                                         start=(k == 0), stop=False)
                    nc.tensor.matmul(ps[:], wtk[:, ms], tokT[:, sl],
                                     start=False, stop=(gz is None))
                    if gz is not None:
                        nc.tensor.matmul(ps[:], idr[:], gz[:, m, :], start=False,
                                         stop=True)
                    return ps

                for t in range(NSTEP):
                    for ci in range(NCH):
                        sl = slice(ci * NC, (ci + 1) * NC)
                        # one batched gz load for all 12 gate row chunks
                        gz = gzp.tile([128, MC, NC], F32R, tag="gz")
                        nc.gpsimd.dma_start(
                            out=gz[:], in_=gz_d[:, :, sl].rearrange("m p b -> p m b"))
                        # r and n gates interleaved per row-chunk a:
                        # r rows 0:512 (m 0..3), n rows 1024:1536 (m 8..11)
                        nt = nwk.tile([128, 4, NC], F32, tag="n")
                        for a in range(4):
                            ps_r = gate_psum(a, ci, gz)
                            r_a = wk.tile([128, NC], F32, tag="r")
                            nc.scalar.activation(r_a[:], ps_r[:], AF.Sigmoid,
                                                 bias=brz_s[:, a:a + 1], scale=1.0)
                            m = 8 + a
                            sm = slice(m * 128, (m + 1) * 128)
                            ps_hn = pg.tile([128, NC], F32, tag="gate")
                            for k in range(KH):
                                nc.tensor.matmul(ps_hn[:], whh[:, k, sm], hT[:, k, sl],
                                                 start=(k == 0), stop=(k == KH - 1))
                            ps_in = pg.tile([128, NC], F32, tag="gate")
                            nc.tensor.matmul(ps_in[:], wtk[:, sm], tokT[:, sl],
                                             start=True, stop=True)
                            # i_n = tok + btanh + gz ; npre = (gh_n+b_hh_n)*r
                            i_n = wk.tile([128, NC], F32, tag="r")
                            nc.vector.scalar_tensor_tensor(
                                i_n[:], ps_in[:], btn_s[:, a:a + 1],
                                gz[:, m, :].bitcast(F32),
                                AluOpType.add, AluOpType.add)
                            npre = wk.tile([128, NC], F32, tag="npre")
                            nc.vector.scalar_tensor_tensor(
                                npre[:], ps_hn[:], bhn_s[:, a:a + 1], r_a[:],
                                AluOpType.add, AluOpType.mult)
                            nc.vector.tensor_tensor(npre[:], npre[:], i_n[:],
                                                    AluOpType.add)
                            nc.scalar.activation(nt[:, a, :], npre[:], AF.Tanh)
                        # z gates (rows 512:1024 -> m 4..7): ALL must read the
                        # old h before any h write below
                        zt = nwk.tile([128, 4, NC], F32, tag="z4")
                        for a in range(4):
                            ps_z = gate_psum(4 + a, ci, gz)
                            nc.scalar.activation(zt[:, a, :], ps_z[:], AF.Sigmoid,
                                                 bias=brz_s[:, 4 + a:5 + a], scale=1.0)
                        # h' = n + z * (h - n)
                        for a in range(4):
                            u_a = wk.tile([128, NC], F32, tag="r")
                            nc.gpsimd.tensor_tensor(u_a[:], hT[:, a, sl].bitcast(F32),
                                                    nt[:, a, :], AluOpType.subtract)
                            zd_a = wk.tile([128, NC], F32, tag="npre")
                            nc.vector.tensor_tensor(zd_a[:], zt[:, a, :], u_a[:],
                                                    AluOpType.mult)
                            nc.vector.tensor_tensor(hT[:, a, sl], nt[:, a, :], zd_a[:],
                                                    AluOpType.add)

                        # heads: transposed [48, NC] with whd stationary
                        # (f32r full-rate N=512), bias per-partition, then PE
                        # transposes back to batch-major [128, 48] tiles
                        pst_h = ph.tile([48, NC], F32, tag="hpsum")
                        for k in range(KH):
                            nc.tensor.matmul(pst_h[:], whd[:, k, :],
                                             hT[:, k, sl], start=(k == 0),
                                             stop=(k == KH - 1))
                        hds = hdp.tile([48, NC], F32, tag="hds")
                        nc.scalar.activation(hds[:], pst_h[:], AF.Identity,
                                             bias=hb48[:, 0:1], scale=1.0)
                        psb = ph.tile([128, BT, 48], F32, tag="hpsum2")
                        for j in range(BT):
                            nc.tensor.transpose(psb[:, j, :],
                                                hds[:, j * 128:(j + 1) * 128],
                                                idr[0:48, 0:48].bitcast(F32))
                        HD = hdp.tile([128, BT, 48], F32, tag="hd")
                        nc.scalar.copy(HD[:], psb[:])
                        # stream outputs (t-major DRAM layout)
                        nc.sync.dma_start(
                            out=roots_o[t, sl, :].rearrange("(j p) c -> p j c", p=128),
                            in_=HD[:, :, 0:12])
                        nc.sync.dma_start(
                            out=chroma_o[t, sl, :].rearrange("(j p) c -> p j c", p=128),
                            in_=HD[:, :, 12:36])
                        nc.sync.dma_start(
                            out=bass_o[t, sl, :].rearrange("(j p) c -> p j c", p=128),
                            in_=HD[:, :, 36:48])

                        if t == NSTEP - 1:
                            continue
                        # next token: one-hot(argmax root/bass) + chroma bits
                        tkb = hdp.tile([128, BT, TOK], BF16, tag="tkb")
                        mxr = hdp.tile([128, BT], F32, tag="mxr")
                        mxb = hdp.tile([128, BT], F32, tag="mxb")
                        nc.vector.tensor_reduce(mxr[:], HD[:, :, 0:12], AX.X,
                                                AluOpType.max)
                        nc.vector.tensor_reduce(mxb[:], HD[:, :, 36:48], AX.X,
                                                AluOpType.max)
                        nc.vector.tensor_tensor(
                            tkb[:, :, 0:12], HD[:, :, 0:12],
                            mxr[:].broadcast_to([128, BT, 12]), AluOpType.is_ge)
                        nc.vector.tensor_tensor(
                            tkb[:, :, 24:36], HD[:, :, 36:48],
                            mxb[:].broadcast_to([128, BT, 12]), AluOpType.is_ge)
                        pairs = HD[:, :, 12:36].rearrange(
                            "p j (c two) -> p j c two", two=2)
                        nc.vector.tensor_tensor(
                            tkb[:, :, 12:24], pairs[:, :, :, 1], pairs[:, :, :, 0],
                            AluOpType.is_gt)
                        # transpose token back to [36, NC] (bf16 PE transpose)
                        pstk = pt.tile([TOK, NC], BF16, tag="tk")
                        for j in range(BT):
                            nc.tensor.transpose(pstk[:, j * 128:(j + 1) * 128],
                                                tkb[:, j, :], idb[:])
                        nc.vector.tensor_copy(tokT[:, sl], pstk[:])

    nc.finalize()
    return nc


def _host_prep(inputs):
    f = np.float32
    W_ih = np.asarray(inputs["W_ih"], f)
    W_hh = np.asarray(inputs["W_hh"], f)
    b_ih = np.asarray(inputs["b_ih"], f)
    b_hh = np.asarray(inputs["b_hh"], f)
    z2in_w = np.asarray(inputs["z2dec_in_w"], f)
    z2in_b = np.asarray(inputs["z2dec_in_b"], f)
    z2hid_w = np.asarray(inputs["z2dec_hid_w"], f)
    z2hid_b = np.asarray(inputs["z2dec_hid_b"], f)
    W_tok = W_ih[:, :TOK]
    W_z = W_ih[:, TOK:]
    M_gz = (W_z @ z2in_w).astype(f)
    c_gz = (W_z @ z2in_b + b_ih).astype(f)

    wheads = np.concatenate([inputs["root_w"], inputs["chroma_w"],
                             inputs["bass_w"]], 0).astype(f)    # [48, 512]
    hb = np.concatenate([inputs["root_b"], inputs["chroma_b"],
                         inputs["bass_b"]], 0).astype(f)        # [48]

    import ml_dtypes
    common = {
        "whhT": np.ascontiguousarray(W_hh.T),
        "wtokT": np.ascontiguousarray(W_tok.T),
        "wgzT": np.ascontiguousarray(M_gz.T),
        "whidT": np.ascontiguousarray(z2hid_w.T),
        "wheadT": np.ascontiguousarray(wheads.T),
        "hbias": hb.reshape(1, 48),
        "brz": b_hh[:2 * H].copy(),
        "bhn": b_hh[2 * H:].copy(),
        "btanh": np.zeros(H, f),
        "bh0": z2hid_b,
        "cgz": c_gz,
        "ident": np.eye(128, dtype=f),
        "ident_b": np.eye(128).astype(ml_dtypes.bfloat16),
        "tok0T": np.ascontiguousarray(
            np.broadcast_to(np.asarray(inputs["init_input"], f)[:, None],
                            (TOK, BSH))),
    }
    z_chd = np.asarray(inputs["z_chd"], f)
    in_maps = []
    for i in range(NCORES):
        m = dict(common)
        m["z_chdT"] = np.ascontiguousarray(z_chd[i * BSH:(i + 1) * BSH, :].T)
        in_maps.append(m)
    return in_maps


def kernel(**inputs):
    global _BUILT
    if _BUILT is None:
        _BUILT = _build()
    nc = _BUILT
    in_maps = _host_prep(inputs)
    res = run_bass_kernel_spmd(nc, in_maps, list(range(NCORES))).results

    roots = np.concatenate([r["roots_o"].transpose(1, 0, 2) for r in res], 0)
    chroma = np.concatenate([r["chroma_o"].transpose(1, 0, 2) for r in res], 0)
    basses = np.concatenate([r["bass_o"].transpose(1, 0, 2) for r in res], 0)
    return (roots.astype(np.float32),
            chroma.reshape(BS, NSTEP, 12, 2).astype(np.float32),
            basses.astype(np.float32))
